# revision 1
# baseline (speedup 1.0000x reference)
"""AttentionPool2d (sparse attention) on 8 Trainium2 NeuronCores via Bass/Tile.

Self-contained: builds an 8-core SPMD Bass program (shard over the pixel/L
dimension, sequence-parallel softmax with two AllReduces), compiles once per
process, and runs via the axon PJRT path.

Math (reference):
  xs   = x.reshape(C, HW).T                      [HW, C]
  m    = sigmoid(masks).reshape(Q, HW).T         [HW, Q]
  mean = (m.T @ xs) / (m.sum(0) + 1e-3)          [Q, C]
  seq  = [mean; xs]                              [L, C]
  q,k,v = linear projections; q scaled by hd^-.5
  attn mask: pooled queries attend only to self among pooled tokens (eye)
  and to pixels with sigmoid > 0.9; softmax over L; out = ctx @ Wc.T + bc.

Distribution: core i owns pixels [2048*i, 2048*(i+1)). All cores process all
200 pooled tokens with the diagonal mask scaled by 1/8 (the final AllReduce
adds the 8 identical copies back to 1). Softmax runs without max-subtraction
(scores are O(0.1) here; shift-invariance makes any uniform bias exact).
Denominators come from a ones-column appended to v, so ctx and sums travel
in one AllReduce buffer.

Matmul dtype: float32r (TF32 path, full PE rate at moving-dim >= 256) when
USE_F32R, else float32 (4 cycles/row). Tensors consumed by f32r matmuls are
declared float32r end-to-end (walrus requires producers to round to f32r).
"""
import numpy as np

import concourse.bass as bass
import concourse.bacc as bacc
import concourse.mybir as mybir
import concourse.tile as tile
from concourse import masks as masks_mod

F32 = mybir.dt.float32
F32R = mybir.dt.float32r
AF = mybir.ActivationFunctionType
ALU = mybir.AluOpType

NCORES = 8
C = 1024          # embed dim
NH = 16           # heads
HD = 64           # head dim
Q = 200           # pooled queries
QP = 256          # padded query dim (f32r fast path needs moving dim >= 256)
HW = 128 * 128
LPIX = HW // NCORES   # 2048 pixels per core
NSC = LPIX // 128     # 16 l-subchunks in phase A
NDC = LPIX // 256     # 8 double-chunks in attention phase
EXP_BIAS = 0.0        # uniform shift inside exp(); cancels in softmax

USE_F32R = True
MDT = F32R if USE_F32R else F32   # dtype of every matmul operand


def _ms(nc, ap, v):
    # Memset has no f32r encoding; the bit pattern is identical to f32.
    nc.vector.memset(ap.bitcast(F32) if ap.dtype == F32R else ap, v)


def build(phases=3, nsc=NSC, ndc=NDC):
    nc = bacc.Bacc("TRN2", target_bir_lowering=False, debug=False,
                   num_devices=NCORES)

    xsr_d = nc.dram_tensor("xsr", [128, NDC, 8, 256], MDT, kind="ExternalInput")
    xtr_d = nc.dram_tensor("xtr", [128, NSC, C], MDT, kind="ExternalInput")
    mskt_d = nc.dram_tensor("mskt", [128, NSC, Q], F32, kind="ExternalInput")
    m01_d = nc.dram_tensor("m01", [128, NSC, Q], F32, kind="ExternalInput")
    wkt_d = nc.dram_tensor("wkt", [128, 8, C], MDT, kind="ExternalInput")
    wvt_d = nc.dram_tensor("wvt", [128, 8, C], MDT, kind="ExternalInput")
    wqt_d = nc.dram_tensor("wqt", [128, 8, C], MDT, kind="ExternalInput")
    wct_d = nc.dram_tensor("wct", [128, 8, 128], MDT, kind="ExternalInput")
    bk_d = nc.dram_tensor("bk", [128, 8], F32, kind="ExternalInput")
    bq_d = nc.dram_tensor("bq", [128, 8], F32, kind="ExternalInput")
    bvr_d = nc.dram_tensor("bvr", [1, C], MDT, kind="ExternalInput")
    bc_d = nc.dram_tensor("bc", [128, 1], F32, kind="ExternalInput")
    diag_d = nc.dram_tensor("diag", [Q, Q], F32, kind="ExternalInput")
    sel_d = nc.dram_tensor("sel", [2, 128], MDT, kind="ExternalInput")
    onesm_d = nc.dram_tensor("onesm", [128, 128], MDT, kind="ExternalInput")
    zpad_d = nc.dram_tensor("zpad", [128, 8 * (QP - Q)], MDT, kind="ExternalInput")
    outp_d = nc.dram_tensor("outp", [128, Q], F32, kind="ExternalOutput")

    wkt_r = wkt_d.ap()
    wvt_r = wvt_d.ap()
    wqt_r = wqt_d.ap()
    wct_r = wct_d.ap()
    bk_r = bk_d.ap()
    bq_r = bq_d.ap()

    RG = [list(range(NCORES))]

    with tile.TileContext(nc) as tc:
        with (
            tc.tile_pool(name="const", bufs=1) as cst,
            tc.tile_pool(name="pers", bufs=1) as pers,
            tc.tile_pool(name="drp", bufs=1, space="DRAM") as drp,
        ):
            # DMA issue order matters for startup latency: the first pooling
            # matmul needs only onesm + the first xtr/mskt chunks, so issue
            # the small constants first and the big weight loads last.
            onesm = cst.tile([128, 128], MDT)
            nc.sync.dma_start(onesm[:], onesm_d.ap())
            ones_col = onesm[:, 0:1]
            ones_row = onesm[0:1, :]
            bk_sb = cst.tile([128, 8], F32)
            nc.sync.dma_start(bk_sb[:], bk_r)
            bq_sb = cst.tile([128, 8], F32)
            nc.sync.dma_start(bq_sb[:], bq_r)
            bvr_sb = cst.tile([1, C], MDT)
            nc.sync.dma_start(bvr_sb[:], bvr_d.ap())
            bc_sb = cst.tile([128, 1], F32)
            nc.sync.dma_start(bc_sb[:], bc_d.ap())
            diag0 = cst.tile([128, Q], F32)
            nc.sync.dma_start(diag0[:], diag_d.ap()[0:128, :])
            diag1 = cst.tile([72, Q], F32)
            nc.sync.dma_start(diag1[:], diag_d.ap()[128:Q, :])
            # selector for broadcasting head-sums to 64-row halves:
            # sel.T @ [s_even; s_odd] -> rows 0:64 = s_even, 64:128 = s_odd
            sel = cst.tile([2, 128], MDT)
            nc.sync.dma_start(sel[:], sel_d.ap())
            ident = cst.tile([128, 128], F32)
            masks_mod.make_identity(nc, ident[:])
            # big weight loads go on engines that are idle during phase A so
            # the SP stream can issue the first xtr/mskt chunk DMAs at once
            wk_sb = cst.tile([128, 8, C], MDT)
            nc.gpsimd.dma_start(wk_sb[:], wkt_r)
            wv_sb = cst.tile([128, 8, C], MDT)
            nc.gpsimd.dma_start(wv_sb[:], wvt_r)
            wc_sb = cst.tile([128, 8, 128], MDT)
            nc.gpsimd.dma_start(wc_sb[:], wct_r)

            # survive across phases
            mask01 = pers.tile([128, NSC, Q], F32)
            qt_sb = pers.tile([128, 8, QP], MDT)
            ctx_sb = pers.tile([65, NH, Q], F32)
            # pixel k/v for the first NPRE dc-chunks, computed while AR1 is
            # in flight (they depend only on x and Wk/Wv, not on the mean)
            NPRE = 1
            kt_pre = [pers.tile([128, 8, 256], MDT, name=f"kt_pre{i}")
                      for i in range(NPRE)]
            vt_pre = [[pers.tile([128, NH * 65], MDT, name=f"vt_pre{i}_{j}")
                       for j in range(2)] for i in range(NPRE)]

            ar1i = drp.tile([Q + 1, C], F32)
            ar1o = drp.tile([Q + 1, C], F32, addr_space="Shared")
            ar2i = drp.tile([C + NH, Q], F32)
            ar2o = drp.tile([C + NH, Q], F32, addr_space="Shared")

            # ---------------- Phase A: sigmoid + pooling -------------------
            # (x and masks arrive host-pre-transposed; mask bits host-computed)
            nc.gpsimd.dma_start(mask01[:], m01_d.ap())
            with (
                tc.tile_pool(name="pAs", bufs=2) as pAs,
                tc.tile_pool(name="psA", bufs=1, space="PSUM") as psA,
            ):
                # pooling accumulators: mean partial, [q, c] layout
                pm00 = psA.tile([128, 512], F32, tag="pm00")
                pm01 = psA.tile([128, 512], F32, tag="pm01")
                pm10 = psA.tile([72, 512], F32, tag="pm10")
                pm11 = psA.tile([72, 512], F32, tag="pm11")
                pw = psA.tile([1, Q], F32, tag="pw")

                for sc in range(nsc):
                    xT = pAs.tile([128, C], MDT, tag="xT")
                    nc.sync.dma_start(xT[:], xtr_d.ap()[:, sc, :])
                    mraw = pAs.tile([128, Q], F32, tag="mraw")
                    nc.sync.dma_start(mraw[:], mskt_d.ap()[:, sc, :])
                    mT = pAs.tile([128, Q], MDT, tag="mT")
                    nc.scalar.activation(mT[:], mraw[:], AF.Sigmoid)

                    st, sp = (sc == 0), (sc == nsc - 1)
                    nc.tensor.matmul(pm00[:], mT[:, 0:128], xT[:, 0:512], start=st, stop=sp)
                    nc.tensor.matmul(pm01[:], mT[:, 0:128], xT[:, 512:1024], start=st, stop=sp)
                    nc.tensor.matmul(pm10[:], mT[:, 128:Q], xT[:, 0:512], start=st, stop=sp)
                    nc.tensor.matmul(pm11[:], mT[:, 128:Q], xT[:, 512:1024], start=st, stop=sp)
                    # w partial: ones.T @ mT -> [1, Q]
                    nc.tensor.matmul(pw[:], ones_col, mT[:], start=st, stop=sp)

                # stage AR1 input (PSUM -> SBUF -> DRAM)
                mean0 = pAs.tile([128, C], F32, bufs=1)
                nc.any.tensor_copy(mean0[:, 0:512], pm00[:])
                nc.any.tensor_copy(mean0[:, 512:1024], pm01[:])
                mean1 = pAs.tile([72, C], F32, bufs=1)
                nc.any.tensor_copy(mean1[:, 0:512], pm10[:])
                nc.any.tensor_copy(mean1[:, 512:1024], pm11[:])
                nc.sync.dma_start(ar1i[0:128, :], mean0[:])
                nc.sync.dma_start(ar1i[128:Q, :], mean1[:])
                wrow = pAs.tile([1, C], F32, bufs=1)
                nc.vector.memset(wrow[:], 0.0)
                nc.vector.tensor_copy(wrow[0:1, 0:Q], pw[:])
                nc.sync.dma_start(ar1i[Q:Q + 1, :], wrow[:])

            nc.gpsimd.collective_compute(
                "AllReduce", ALU.add, replica_groups=RG,
                ins=[ar1i.opt()], outs=[ar1o.opt()],
            )

            if phases == 1:
                with tc.tile_pool(name="pX", bufs=1) as pX:
                    ob = pX.tile([128, Q], F32)
                    nc.sync.dma_start(ob[:], ar1o[0:128, 0:Q])
                    nc.sync.dma_start(outp_d.ap(), ob[:])
                nc.compile()
                return nc

            # ---- Phase C prelude: pixel k/v for the first NPRE dc-chunks.
            # Emitted right after the AR1 launch so the PE chews on work
            # that doesn't depend on the pooled mean while the collective
            # and the phase-B dependency chain (meang DMA, reciprocals,
            # transposes) are in flight.
            with (
                tc.tile_pool(name="pPre", bufs=2) as pPre,
                tc.tile_pool(name="psPre", bufs=1, space="PSUM") as psPre,
            ):
                for dc in range(NPRE):
                    x_dc = pPre.tile([128, 8, 256], MDT, tag="xdc")
                    nc.sync.dma_start(x_dc[:], xsr_d.ap()[:, dc, :, :])
                    for a in range(8):
                        pk = psPre.tile([128, 256], F32, tag="pkt", bufs=2)
                        for kc in range(8):
                            nc.tensor.matmul(pk[:], wk_sb[:, kc, a * 128:(a + 1) * 128],
                                             x_dc[:, kc, :],
                                             start=(kc == 0), stop=(kc == 7))
                        nc.any.tensor_scalar_add(kt_pre[dc][:, a, :], pk[:], bk_sb[:, a:a + 1])
                    for ls in range(2):
                        vr = vt_pre[dc][ls][:].rearrange("p (h e) -> p h e", e=65)
                        for nn in range(2):
                            pv = psPre.tile([128, 512], F32, tag="pv", bufs=2)
                            for kc in range(8):
                                nc.tensor.matmul(pv[:], x_dc[:, kc, ls * 128:(ls + 1) * 128],
                                                 wv_sb[:, kc, nn * 512:(nn + 1) * 512],
                                                 start=(kc == 0), stop=False)
                            nc.tensor.matmul(pv[:], onesm[0:1, :],
                                             bvr_sb[0:1, nn * 512:(nn + 1) * 512],
                                             start=False, stop=True)
                            nc.any.tensor_copy(
                                vr[:, nn * 8:(nn + 1) * 8, 0:64],
                                pv[:].rearrange("p (h e) -> p h e", e=64))
                        nc.vector.tensor_copy(vr[:, :, 64:65], onesm[:, 0:NH].unsqueeze(2))

            # ------------- Phase B: mean scaling, qT, mean-token k/v -------
            with (
                tc.tile_pool(name="pB", bufs=1) as pB,
                tc.tile_pool(name="pBs", bufs=2) as pBs,
                tc.tile_pool(name="psB", bufs=1, space="PSUM") as psB,
            ):
                wq_sb = pB.tile([128, 8, C], MDT)
                nc.sync.dma_start(wq_sb[:], wqt_r)
                meang0 = pB.tile([128, C], F32)
                nc.sync.dma_start(meang0[:], ar1o[0:128, :])
                meang1 = pB.tile([72, C], F32)
                nc.sync.dma_start(meang1[:], ar1o[128:Q, :])
                # w row -> per-partition column. A transposed-view DMA would
                # emit 200 single-element descriptors (~13us on the SP queue,
                # serializing the whole post-AR1 chain); a PE transpose of
                # the contiguous row costs ~1us.
                wrow_g = pB.tile([1, Q], F32)
                nc.sync.dma_start(wrow_g[:], ar1o[Q:Q + 1, 0:Q])
                pt0 = psB.tile([128, 128], F32, tag="tp", bufs=2)
                nc.tensor.transpose(pt0[:, 0:1], wrow_g[0:1, 0:128], ident[0:1, 0:1])
                rw0 = pB.tile([128, 1], F32)
                nc.vector.tensor_scalar_add(rw0[:], pt0[:, 0:1], 0.001)
                nc.vector.reciprocal(rw0[:], rw0[:])
                pt1 = psB.tile([128, 128], F32, tag="tp", bufs=2)
                nc.tensor.transpose(pt1[0:72, 0:1], wrow_g[0:1, 128:Q], ident[0:1, 0:1])
                rw1 = pB.tile([72, 1], F32)
                nc.vector.tensor_scalar_add(rw1[:], pt1[0:72, 0:1], 0.001)
                nc.vector.reciprocal(rw1[:], rw1[:])

                msc0 = pB.tile([128, C], F32)
                nc.vector.tensor_scalar_mul(msc0[:], meang0[:], rw0[:])
                msc1 = pB.tile([72, C], F32)
                nc.vector.tensor_scalar_mul(msc1[:], meang1[:], rw1[:])

                # meanT [c, q] with zero-padded q cols
                meanT = pB.tile([128, 8, QP], MDT)
                nc.sync.dma_start(
                    meanT[:, :, Q:QP],
                    zpad_d.ap().rearrange("p (a z) -> p a z", a=8))
                for a in range(8):
                    t0 = psB.tile([128, 128], F32, tag="tp", bufs=2)
                    nc.tensor.transpose(t0[:], msc0[:, a * 128:(a + 1) * 128], ident[:])
                    nc.any.tensor_copy(meanT[:, a, 0:128], t0[:])
                    t1 = psB.tile([128, 128], F32, tag="tp", bufs=2)
                    nc.tensor.transpose(t1[:, 0:72], msc1[:, a * 128:(a + 1) * 128], ident[0:72, 0:72])
                    nc.any.tensor_copy(meanT[:, a, 128:Q], t1[:, 0:72])

                # qT and kT over mean tokens
                ktm = pB.tile([128, 8, Q], MDT)
                for a in range(8):
                    pq = psB.tile([128, QP], F32, tag="pq", bufs=2)
                    for kc in range(8):
                        nc.tensor.matmul(pq[:], wq_sb[:, kc, a * 128:(a + 1) * 128],
                                         meanT[:, kc, :],
                                         start=(kc == 0), stop=(kc == 7))
                    nc.any.tensor_scalar_add(qt_sb[:, a, :], pq[:], bq_sb[:, a:a + 1])
                    pk = psB.tile([128, QP], F32, tag="pq", bufs=2)
                    for kc in range(8):
                        nc.tensor.matmul(pk[:], wk_sb[:, kc, a * 128:(a + 1) * 128],
                                         meanT[:, kc, :],
                                         start=(kc == 0), stop=(kc == 7))
                    nc.any.tensor_scalar_add(ktm[:, a, :], pk[:, 0:Q], bk_sb[:, a:a + 1])

                # v over mean tokens, with ones column per head
                vm0 = pB.tile([128, NH * 65], MDT)
                vm1 = pB.tile([72, NH * 65], MDT)
                vm0r = vm0[:].rearrange("p (h e) -> p h e", e=65)
                vm1r = vm1[:].rearrange("p (h e) -> p h e", e=65)
                for (vt, mw, PQC) in ((vm0r, 128, slice(0, 128)), (vm1r, 72, slice(128, Q))):
                    for nn in range(2):
                        pv = psB.tile([128, 512], F32, tag="pq", bufs=2)
                        for kc in range(8):
                            nc.tensor.matmul(pv[0:mw, :], meanT[:, kc, PQC],
                                             wv_sb[:, kc, nn * 512:(nn + 1) * 512],
                                             start=(kc == 0), stop=False)
                        nc.tensor.matmul(pv[0:mw, :], onesm[0:1, 0:mw],
                                         bvr_sb[0:1, nn * 512:(nn + 1) * 512],
                                         start=False, stop=True)
                        nc.any.tensor_copy(
                            vt[:, nn * 8:(nn + 1) * 8, 0:64],
                            pv[0:mw, :].rearrange("p (h e) -> p h e", e=64))
                    nc.vector.tensor_copy(vt[:, :, 64:65], onesm[0:mw, 0:NH].unsqueeze(2))

                # mean-token attention block (all 200 tokens, diag/8 mask).
                # scores go through per-head single-bank PSUM tiles: a shared
                # multi-bank PSUM tile with several matmul groups wedges the
                # device (found empirically).
                for g in range(4):
                    pbm = []
                    for (qc, P, dg) in ((0, 128, diag0), (1, 72, diag1)):
                        pr = pBs.tile([128, 4, QP], MDT, tag="pbm")
                        for hh in range(4):
                            h = 4 * g + hh
                            a, r0 = h // 2, 64 * (h % 2)
                            ps1 = psB.tile([128, QP], F32, tag="ss1", bufs=2)
                            nc.tensor.matmul(
                                ps1[0:P, :],
                                ktm[r0:r0 + 64, a, qc * 128:qc * 128 + P],
                                qt_sb[r0:r0 + 64, a, :],
                                start=True, stop=True)
                            nc.scalar.activation(pr[0:P, hh, :], ps1[0:P, :], AF.Exp, bias=EXP_BIAS)
                        mb = dg[0:P, :].unsqueeze(1).broadcast_to([P, 4, Q])
                        nc.gpsimd.tensor_tensor(pr[0:P, :, 0:Q], pr[0:P, :, 0:Q], mb, op=ALU.mult)
                        pbm.append(pr)
                    for hh in range(4):
                        h = 4 * g + hh
                        pc2 = psB.tile([65, QP], F32, tag="sctx", bufs=2)
                        nc.tensor.matmul(pc2[:], vm0r[:, h, :], pbm[0][0:128, hh, :], start=True, stop=False)
                        nc.tensor.matmul(pc2[:], vm1r[:, h, :], pbm[1][0:72, hh, :], start=False, stop=True)
                        nc.any.tensor_copy(ctx_sb[:, h, :], pc2[:, 0:Q])

            if phases == 2:
                with tc.tile_pool(name="pX", bufs=1) as pX:
                    ob = pX.tile([128, Q], F32)
                    nc.vector.tensor_copy(ob[:], qt_sb[:, 0, 0:Q])
                    nc.sync.dma_start(outp_d.ap(), ob[:])
                nc.compile()
                return nc

            # ---------------- Phase C: pixel attention --------------------
            with (
                tc.tile_pool(name="pC", bufs=2) as pC,
                tc.tile_pool(name="pCb", bufs=2) as pCb,
                tc.tile_pool(name="psC", bufs=1, space="PSUM") as psC,
            ):
                for dc in range(ndc):
                    if dc < NPRE:
                        kt = kt_pre[dc]
                        vts = [vt_pre[dc][ls][:].rearrange("p (h e) -> p h e", e=65)
                               for ls in range(2)]
                    else:
                        x_dc = pC.tile([128, 8, 256], MDT, tag="xdc")
                        nc.sync.dma_start(x_dc[:], xsr_d.ap()[:, dc, :, :])
                        kt = pC.tile([128, 8, 256], MDT, tag="kt")
                        for a in range(8):
                            pk = psC.tile([128, 256], F32, tag="pkt", bufs=1)
                            for kc in range(8):
                                nc.tensor.matmul(pk[:], wk_sb[:, kc, a * 128:(a + 1) * 128],
                                                 x_dc[:, kc, :],
                                                 start=(kc == 0), stop=(kc == 7))
                            nc.any.tensor_scalar_add(kt[:, a, :], pk[:], bk_sb[:, a:a + 1])

                        vts = []
                        for ls in range(2):
                            vt = pC.tile([128, NH * 65], MDT, tag=f"vt{ls}", bufs=1)
                            vr = vt[:].rearrange("p (h e) -> p h e", e=65)
                            for nn in range(2):
                                pv = psC.tile([128, 512], F32, tag="pv", bufs=2)
                                for kc in range(8):
                                    nc.tensor.matmul(pv[:], x_dc[:, kc, ls * 128:(ls + 1) * 128],
                                                     wv_sb[:, kc, nn * 512:(nn + 1) * 512],
                                                     start=(kc == 0), stop=False)
                                nc.tensor.matmul(pv[:], onesm[0:1, :],
                                                 bvr_sb[0:1, nn * 512:(nn + 1) * 512],
                                                 start=False, stop=True)
                                nc.any.tensor_copy(
                                    vr[:, nn * 8:(nn + 1) * 8, 0:64],
                                    pv[:].rearrange("p (h e) -> p h e", e=64))
                            nc.vector.tensor_copy(vr[:, :, 64:65], onesm[:, 0:NH].unsqueeze(2))
                            vts.append(vr)

                    for g in range(4):
                        pb = []
                        for ls in range(2):
                            pr = pCb.tile([128, 4, QP], MDT, tag="pb")
                            for hh in range(4):
                                h = 4 * g + hh
                                a, r0 = h // 2, 64 * (h % 2)
                                ps1 = psC.tile([128, QP], F32, tag="ss1", bufs=3)
                                nc.tensor.matmul(
                                    ps1[:],
                                    kt[r0:r0 + 64, a, ls * 128:(ls + 1) * 128],
                                    qt_sb[r0:r0 + 64, a, :],
                                    start=True, stop=True)
                                nc.scalar.activation(pr[:, hh, :], ps1[:], AF.Exp, bias=EXP_BIAS)
                            mb = mask01[:, 2 * dc + ls, :].unsqueeze(1).broadcast_to([128, 4, Q])
                            nc.gpsimd.tensor_tensor(pr[:, :, 0:Q], pr[:, :, 0:Q], mb, op=ALU.mult)
                            pb.append(pr)
                        for hh in range(4):
                            h = 4 * g + hh
                            pc2 = psC.tile([65, QP], F32, tag="sctx", bufs=2)
                            nc.tensor.matmul(pc2[:], vts[0][:, h, :], pb[0][:, hh, :], start=True, stop=False)
                            nc.tensor.matmul(pc2[:], vts[1][:, h, :], pb[1][:, hh, :], start=False, stop=True)
                            nc.vector.tensor_tensor(ctx_sb[:, h, :], ctx_sb[:, h, :], pc2[:, 0:Q], op=ALU.add)

                # ---------------- AR2 + output ----------------------------
                nc.sync.dma_start(
                    ar2i[0:C, :].rearrange("(h p) q -> p h q", p=64),
                    ctx_sb[0:64, :, :])
                # head-sum rows: ctx_sb[64, :, :] is one contiguous [16, 200]
                # block -> single DMA instead of 16 one-row descriptors on
                # the fully-exposed pre-AR2 critical path
                nc.sync.dma_start(ar2i[C:C + NH, :], ctx_sb[64:65, :, :])

                nc.gpsimd.collective_compute(
                    "AllReduce", ALU.add, replica_groups=RG,
                    ins=[ar2i.opt()], outs=[ar2o.opt()],
                )

                ctxg = pC.tile([128, 8, Q], F32, bufs=1)
                nc.sync.dma_start(ctxg[:], ar2o[0:C, :].rearrange("(a p) q -> p a q", p=128))
                sums2 = pC.tile([2, 8, Q], F32, bufs=1)
                nc.sync.dma_start(sums2[:], ar2o[C:C + NH, :].rearrange("(a two) q -> two a q", two=2))
                rsum2 = pC.tile([2, 8, Q], MDT, bufs=1)
                with nc.allow_low_precision(reason="softmax denominators; f32r rounding is the chosen matmul precision"):
                    nc.vector.reciprocal(rsum2[:], sums2[:])
                ctxn = pC.tile([128, 8, Q], MDT, bufs=1)
                for a in range(8):
                    prb = psC.tile([128, 512], F32, tag="pv", bufs=2)
                    nc.tensor.matmul(prb[:, 0:Q], sel[:], rsum2[:, a, :], start=True, stop=True)
                    nc.vector.tensor_tensor(ctxn[:, a, :], ctxg[:, a, :], prb[:, 0:Q], op=ALU.mult)

                po = psC.tile([128, Q], F32, tag="pkt", bufs=1)
                for kc in range(8):
                    nc.tensor.matmul(po[:], wc_sb[:, kc, :], ctxn[:, kc, :],
                                     start=(kc == 0), stop=(kc == 7))
                outs = pC.tile([128, Q], F32, bufs=1)
                nc.any.tensor_scalar_add(outs[:], po[:], bc_sb[:])
                nc.sync.dma_start(outp_d.ap(), outs[:])

    nc.compile()
    return nc


def make_runner(nc, n_cores=NCORES):
    """Compile nc into a reusable multi-core PJRT callable (compiles once)."""
    import time as _time
    import jax
    from jax.sharding import Mesh, PartitionSpec, NamedSharding
    from jax.experimental.shard_map import shard_map
    from concourse import bass2jax as b2j

    b2j.install_neuronx_cc_hook()

    partition_name = nc.partition_id_tensor.name if nc.partition_id_tensor else None
    in_names, out_names, out_avals, zero_outs = [], [], [], []
    for alloc in nc.m.functions[0].allocations:
        if not isinstance(alloc, mybir.MemoryLocationSet):
            continue
        name = alloc.memorylocations[0].name
        if alloc.kind == "ExternalInput":
            if name != partition_name:
                in_names.append(name)
        elif alloc.kind == "ExternalOutput":
            out_names.append(name)
            shape = tuple(alloc.tensor_shape)
            dtype = mybir.dt.np(alloc.dtype)
            out_avals.append(jax.core.ShapedArray(shape, dtype))
            zero_outs.append(np.zeros(shape, dtype))

    n_params = len(in_names)
    n_outs = len(out_avals)
    all_in_names = in_names + out_names
    if partition_name is not None:
        all_in_names = all_in_names + [partition_name]

    def _body(*args):
        operands = list(args)
        if partition_name is not None:
            operands.append(b2j.partition_id_tensor())
        outs = b2j._bass_exec_p.bind(
            *operands,
            out_avals=tuple(out_avals),
            in_names=tuple(all_in_names),
            out_names=tuple(out_names),
            lowering_input_output_aliases=(),
            sim_require_finite=True,
            sim_require_nnan=True,
            nc=nc,
        )
        return tuple(outs)

    devices = jax.devices()[:n_cores]
    mesh = Mesh(np.asarray(devices), ("core",))
    in_specs = (PartitionSpec("core"),) * (n_params + n_outs)
    out_specs = (PartitionSpec("core"),) * n_outs
    sharded = jax.jit(
        shard_map(_body, mesh=mesh, in_specs=in_specs,
                  out_specs=out_specs, check_rep=False),
        keep_unused=True,
    )
    # Pre-shard args onto the 8 cores. A default device_put would commit
    # everything to one device and force a 232MB resharding inside every
    # timed sharded() call (~25ms/call through the axon tunnel).
    arg_sharding = NamedSharding(mesh, PartitionSpec("core"))

    def run(in_maps, iters=0, debug=False):
        concat_in = [
            np.concatenate([np.asarray(in_maps[c][name]) for c in range(n_cores)], axis=0)
            for name in in_names
        ]
        concat_zeros = [np.zeros((n_cores * z.shape[0], *z.shape[1:]), z.dtype)
                        for z in zero_outs]
        t0 = _time.perf_counter()
        args = [jax.device_put(a, arg_sharding) for a in concat_in + concat_zeros]
        jax.block_until_ready(args)
        if debug:
            tot = sum(a.nbytes for a in concat_in)
            print(f"device_put done: {tot/1e6:.0f} MB in {_time.perf_counter()-t0:.1f}s", flush=True)
        out = sharded(*args)
        jax.block_until_ready(out)
        times = []
        for _ in range(iters):
            t0 = _time.perf_counter()
            out2 = sharded(*args)
            jax.block_until_ready(out2)
            times.append(_time.perf_counter() - t0)
        res = [
            {name: np.asarray(out[i]).reshape(n_cores, *out_avals[i].shape)[c]
             for i, name in enumerate(out_names)}
            for c in range(n_cores)
        ]
        return res, times

    return run


_RUNNER = None


def _get_runner():
    global _RUNNER
    if _RUNNER is None:
        nc = build()
        _RUNNER = make_runner(nc)
    return _RUNNER


def make_in_maps(x, masks, Wq, bq, Wk, bk, Wv, bv, Wc, bc):
    f = lambda a: np.ascontiguousarray(np.asarray(a, dtype=np.float32))
    x, masks = f(x), f(masks)
    Wq, bq, Wk, bk, Wv, bv, Wc, bc = map(f, (Wq, bq, Wk, bk, Wv, bv, Wc, bc))
    X2 = x.reshape(C, HW)
    M2 = masks.reshape(Q, HW)
    s = HD ** -0.5
    WqT = np.ascontiguousarray((Wq * s).T)
    bq_s = f(bq * s)
    WkT = np.ascontiguousarray(Wk.T)
    WvT = np.ascontiguousarray(Wv.T)
    WcT = np.ascontiguousarray(Wc.T)

    def chunked(w):   # [C, N] -> [128, 8, N] with row 128*kc+p -> [p, kc]
        return np.ascontiguousarray(w.reshape(8, 128, -1).transpose(1, 0, 2))

    wkt_h, wvt_h, wqt_h = chunked(WkT), chunked(WvT), chunked(WqT)
    LOGIT09 = np.float32(np.log(9.0))   # sigmoid(x) > 0.9  <=>  x > ln 9
    diag = np.zeros((Q, Q), np.float32)
    diag[np.arange(Q), np.arange(Q)] = 1.0 / NCORES
    selmat = np.zeros((2, 128), np.float32)
    selmat[0, 0:64] = 1.0
    selmat[1, 64:128] = 1.0
    onesm = np.ones((128, 128), np.float32)
    zpad = np.zeros((128, 8 * (QP - Q)), np.float32)
    in_maps = []
    for c in range(NCORES):
        xc = X2[:, c * LPIX:(c + 1) * LPIX]                    # [C, LPIX]
        # xsr[p, dc, kc, l] = x[128*kc+p, 256*dc+l]
        xsr = np.ascontiguousarray(
            xc.reshape(8, 128, NDC, 256).transpose(1, 2, 0, 3))
        # xtr[p, sc, cc] = x[cc, 128*sc+p]
        xtr = np.ascontiguousarray(
            xc.reshape(C, NSC, 128).transpose(2, 1, 0))
        mc = M2[:, c * LPIX:(c + 1) * LPIX]                    # [Q, LPIX]
        mskt = np.ascontiguousarray(mc.reshape(Q, NSC, 128).transpose(2, 1, 0))
        m01 = (mskt > LOGIT09).astype(np.float32)
        in_maps.append({
            "xsr": xsr, "xtr": xtr, "mskt": mskt, "m01": m01,
            "wkt": wkt_h, "wvt": wvt_h, "wqt": wqt_h,
            "wct": chunked(np.ascontiguousarray(WcT[:, c * 128:(c + 1) * 128])),
            "bk": np.ascontiguousarray(bk.reshape(8, 128).T),
            "bq": np.ascontiguousarray(bq_s.reshape(8, 128).T),
            "bvr": bv.reshape(1, C),
            "bc": np.ascontiguousarray(bc[c * 128:(c + 1) * 128].reshape(128, 1)),
            "diag": diag,
            "sel": selmat, "onesm": onesm, "zpad": zpad,
        })
    return in_maps


def kernel(x, masks, Wq, bq, Wk, bk, Wv, bv, Wc, bc):
    in_maps = make_in_maps(x, masks, Wq, bq, Wk, bk, Wv, bv, Wc, bc)
    run = _get_runner()
    results, _ = run(in_maps)
    outT = np.concatenate([results[c]["outp"] for c in range(NCORES)], axis=0)
    return np.ascontiguousarray(outT.T).reshape(Q, 1, C).astype(np.float32)



# revision 9
# speedup vs baseline: 161.7952x; 161.7952x over previous
"""AttentionPool2d (sparse attention) on 8 Trainium2 NeuronCores via Bass/Tile.

Self-contained: builds an 8-core SPMD Bass program (shard over the pixel/L
dimension, sequence-parallel softmax with two AllReduces), compiles once per
process, and runs via the axon PJRT path.

Math (reference):
  xs   = x.reshape(C, HW).T                      [HW, C]
  m    = sigmoid(masks).reshape(Q, HW).T         [HW, Q]
  mean = (m.T @ xs) / (m.sum(0) + 1e-3)          [Q, C]
  seq  = [mean; xs]                              [L, C]
  q,k,v = linear projections; q scaled by hd^-.5
  attn mask: pooled queries attend only to self among pooled tokens (eye)
  and to pixels with sigmoid > 0.9; softmax over L; out = ctx @ Wc.T + bc.

Distribution: core i owns pixels [2048*i, 2048*(i+1)). All cores process all
200 pooled tokens with the diagonal mask scaled by 1/8 (the final AllReduce
adds the 8 identical copies back to 1). Softmax runs without max-subtraction
(shift-invariance makes any uniform bias exact; fp32 PSUM holds the range).
Denominators come from a ones-column appended to v, so ctx and sums travel
in one AllReduce buffer.

Matmul dtype: bfloat16 (1 cycle/row on the PE at any moving size; fp32
"HIGH" mode runs 4 cycles/row and float32r was observed to fall back to it
on this compiler). PSUM accumulation stays fp32, as do both AllReduces.
Matmul moving dim must stay <= 512: fp32 PSUM output is limited to one
2KB PSUM bank per matmul.
"""
import numpy as np
import ml_dtypes

import concourse.bass as bass
import concourse.bacc as bacc
import concourse.mybir as mybir
import concourse.tile as tile
from concourse import masks as masks_mod

F32 = mybir.dt.float32
BF16 = mybir.dt.bfloat16
AF = mybir.ActivationFunctionType
ALU = mybir.AluOpType

NCORES = 8
C = 1024          # embed dim
NH = 16           # heads
HD = 64           # head dim
Q = 200           # pooled queries
HW = 128 * 128
LPIX = HW // NCORES   # 2048 pixels per core
NSC = LPIX // 128     # 16 l-subchunks in phase A
NDC = LPIX // 256     # 8 double-chunks in attention phase
EXP_BIAS = 0.0        # uniform shift inside exp(); cancels in softmax

MDT = BF16            # dtype of every matmul operand


def build(phases=3, nsc=NSC, ndc=NDC):
    nc = bacc.Bacc("TRN2", target_bir_lowering=False, debug=False,
                   num_devices=NCORES)

    xsr_d = nc.dram_tensor("xsr", [128, NDC, 8, 256], MDT, kind="ExternalInput")
    xtr_d = nc.dram_tensor("xtr", [128, NSC, C], MDT, kind="ExternalInput")
    mskt_d = nc.dram_tensor("mskt", [128, NSC, Q], MDT, kind="ExternalInput")
    m01_d = nc.dram_tensor("m01", [128, NSC, Q], MDT, kind="ExternalInput")
    wkt_d = nc.dram_tensor("wkt", [128, 8, C], MDT, kind="ExternalInput")
    wvt_d = nc.dram_tensor("wvt", [128, 8, C], MDT, kind="ExternalInput")
    wqt_d = nc.dram_tensor("wqt", [128, 8, C], MDT, kind="ExternalInput")
    wct_d = nc.dram_tensor("wct", [128, 8, 128], MDT, kind="ExternalInput")
    bk_d = nc.dram_tensor("bk", [128, 8], F32, kind="ExternalInput")
    bq_d = nc.dram_tensor("bq", [128, 8], F32, kind="ExternalInput")
    bvr_d = nc.dram_tensor("bvr", [1, C], MDT, kind="ExternalInput")
    bc_d = nc.dram_tensor("bc", [128, 1], F32, kind="ExternalInput")
    diag_d = nc.dram_tensor("diag", [Q, Q], MDT, kind="ExternalInput")
    sel16_d = nc.dram_tensor("sel16", [NH, 8 * 128], MDT, kind="ExternalInput")
    onesm_d = nc.dram_tensor("onesm", [128, 128], MDT, kind="ExternalInput")
    outp_d = nc.dram_tensor("outp", [128, Q], F32, kind="ExternalOutput")

    RG = [list(range(NCORES))]

    with tile.TileContext(nc) as tc:
        with (
            tc.tile_pool(name="const", bufs=1) as cst,
            tc.tile_pool(name="pers", bufs=1) as pers,
            tc.tile_pool(name="drp", bufs=1, space="DRAM") as drp,
        ):
            # DMA issue order matters for startup latency: the first pooling
            # matmul needs only onesm + the first xtr/mskt chunks, so issue
            # the small constants first and the big weight loads last.
            onesm = cst.tile([128, 128], MDT)
            nc.sync.dma_start(onesm[:], onesm_d.ap())
            ones_col = onesm[:, 0:1]
            bk_sb = cst.tile([128, 8], F32)
            nc.sync.dma_start(bk_sb[:], bk_d.ap())
            bq_sb = cst.tile([128, 8], F32)
            nc.sync.dma_start(bq_sb[:], bq_d.ap())
            bvr_sb = cst.tile([1, C], MDT)
            nc.sync.dma_start(bvr_sb[:], bvr_d.ap())
            bc_sb = cst.tile([128, 1], F32)
            nc.sync.dma_start(bc_sb[:], bc_d.ap())
            diag0 = cst.tile([128, Q], MDT)
            nc.sync.dma_start(diag0[:], diag_d.ap()[0:128, :])
            diag1 = cst.tile([72, Q], MDT)
            nc.sync.dma_start(diag1[:], diag_d.ap()[128:Q, :])
            sel16 = cst.tile([NH, 8 * 128], MDT)
            nc.sync.dma_start(sel16[:], sel16_d.ap())
            ident = cst.tile([128, 128], F32)
            masks_mod.make_identity(nc, ident[:])
            # big weight loads go on engines that are idle during phase A so
            # the SP stream can issue the first xtr/mskt chunk DMAs at once;
            # ordered by first use (k/v in the AR1-overlap prelude, then the
            # pixel-attention mask, q at phase B, c only at the very end).
            wk_sb = cst.tile([128, 8, C], MDT)
            nc.gpsimd.dma_start(wk_sb[:], wkt_d.ap())
            wv_sb = cst.tile([128, 8, C], MDT)
            nc.gpsimd.dma_start(wv_sb[:], wvt_d.ap())
            mask01 = pers.tile([128, NSC, Q], MDT)
            nc.gpsimd.dma_start(mask01[:], m01_d.ap())
            wq_sb = cst.tile([128, 8, C], MDT)
            nc.gpsimd.dma_start(wq_sb[:], wqt_d.ap())
            wc_sb = cst.tile([128, 8, 128], MDT)
            nc.gpsimd.dma_start(wc_sb[:], wct_d.ap())

            # survive across phases
            qt_sb = pers.tile([128, 8, Q], MDT)
            ctx_sb = pers.tile([65, NH, Q], F32)
            # pixel k/v for the first NPRE dc-chunks, computed while AR1 is
            # in flight (they depend only on x and Wk/Wv, not on the mean)
            NPRE = 3
            kt_pre = [pers.tile([128, 8, 256], MDT, name=f"kt_pre{i}")
                      for i in range(NPRE)]
            vt_pre = [[pers.tile([128, NH * 65], MDT, name=f"vt_pre{i}_{j}")
                       for j in range(2)] for i in range(NPRE)]

            ar1i = drp.tile([Q + 1, C], F32)
            ar1o = drp.tile([Q + 1, C], F32, addr_space="Shared")
            ar2i = drp.tile([C + NH, Q], F32)
            ar2o = drp.tile([C + NH, Q], F32, addr_space="Shared")

            # ---------------- Phase A: sigmoid + pooling -------------------
            # (x and masks arrive host-pre-transposed; mask bits host-computed)
            with (
                tc.tile_pool(name="pAs", bufs=2) as pAs,
                tc.tile_pool(name="psA", bufs=1, space="PSUM") as psA,
            ):
                # pooling accumulators: mean partial, [q, c] layout
                pm00 = psA.tile([128, 512], F32, tag="pm00")
                pm01 = psA.tile([128, 512], F32, tag="pm01")
                pm10 = psA.tile([72, 512], F32, tag="pm10")
                pm11 = psA.tile([72, 512], F32, tag="pm11")
                pw = psA.tile([1, Q], F32, tag="pw")

                for sc in range(nsc):
                    xT = pAs.tile([128, C], MDT, tag="xT")
                    nc.sync.dma_start(xT[:], xtr_d.ap()[:, sc, :])
                    mraw = pAs.tile([128, Q], MDT, tag="mraw")
                    nc.sync.dma_start(mraw[:], mskt_d.ap()[:, sc, :])
                    mT = pAs.tile([128, Q], MDT, tag="mT")
                    nc.scalar.activation(mT[:], mraw[:], AF.Sigmoid)

                    st, sp = (sc == 0), (sc == nsc - 1)
                    nc.tensor.matmul(pm00[:], mT[:, 0:128], xT[:, 0:512], start=st, stop=sp)
                    nc.tensor.matmul(pm01[:], mT[:, 0:128], xT[:, 512:1024], start=st, stop=sp)
                    nc.tensor.matmul(pm10[:], mT[:, 128:Q], xT[:, 0:512], start=st, stop=sp)
                    nc.tensor.matmul(pm11[:], mT[:, 128:Q], xT[:, 512:1024], start=st, stop=sp)
                    # w partial: ones.T @ mT -> [1, Q]
                    nc.tensor.matmul(pw[:], ones_col, mT[:], start=st, stop=sp)

                # stage AR1 input (PSUM -> SBUF -> DRAM)
                mean0 = pAs.tile([128, C], F32, bufs=1)
                nc.any.tensor_copy(mean0[:, 0:512], pm00[:])
                nc.any.tensor_copy(mean0[:, 512:1024], pm01[:])
                mean1 = pAs.tile([72, C], F32, bufs=1)
                nc.any.tensor_copy(mean1[:, 0:512], pm10[:])
                nc.any.tensor_copy(mean1[:, 512:1024], pm11[:])
                nc.sync.dma_start(ar1i[0:128, :], mean0[:])
                nc.sync.dma_start(ar1i[128:Q, :], mean1[:])
                wrow = pAs.tile([1, C], F32, bufs=1)
                nc.vector.memset(wrow[:], 0.0)
                nc.vector.tensor_copy(wrow[0:1, 0:Q], pw[:])
                nc.sync.dma_start(ar1i[Q:Q + 1, :], wrow[:])

            nc.gpsimd.collective_compute(
                "AllReduce", ALU.add, replica_groups=RG,
                ins=[ar1i.opt()], outs=[ar1o.opt()],
            )

            if phases == 1:
                with tc.tile_pool(name="pX", bufs=1) as pX:
                    ob = pX.tile([128, Q], F32)
                    nc.sync.dma_start(ob[:], ar1o[0:128, 0:Q])
                    nc.sync.dma_start(outp_d.ap(), ob[:])
                nc.compile()
                return nc

            # ---- Phase C prelude: pixel k/v for the first NPRE dc-chunks.
            # Emitted right after the AR1 launch so the PE chews on work
            # that doesn't depend on the pooled mean while the collective
            # and the phase-B dependency chain are in flight.
            with (
                tc.tile_pool(name="pPre", bufs=2) as pPre,
                tc.tile_pool(name="psPre", bufs=1, space="PSUM") as psPre,
            ):
                for dc in range(NPRE):
                    x_dc = pPre.tile([128, 8, 256], MDT, tag="xdc")
                    nc.sync.dma_start(x_dc[:], xsr_d.ap()[:, dc, :, :])
                    for a in range(8):
                        pk = psPre.tile([128, 256], F32, tag="pkt", bufs=2)
                        for kc in range(8):
                            nc.tensor.matmul(pk[:], wk_sb[:, kc, a * 128:(a + 1) * 128],
                                             x_dc[:, kc, :],
                                             start=(kc == 0), stop=(kc == 7))
                        nc.any.tensor_scalar_add(kt_pre[dc][:, a, :], pk[:], bk_sb[:, a:a + 1])
                    for ls in range(2):
                        vr = vt_pre[dc][ls][:].rearrange("p (h e) -> p h e", e=65)
                        for nn in range(2):
                            pv = psPre.tile([128, 512], F32, tag="pv", bufs=2)
                            for kc in range(8):
                                nc.tensor.matmul(pv[:], x_dc[:, kc, ls * 128:(ls + 1) * 128],
                                                 wv_sb[:, kc, nn * 512:(nn + 1) * 512],
                                                 start=(kc == 0), stop=False)
                            nc.tensor.matmul(pv[:], onesm[0:1, :],
                                             bvr_sb[0:1, nn * 512:(nn + 1) * 512],
                                             start=False, stop=True)
                            nc.any.tensor_copy(
                                vr[:, nn * 8:(nn + 1) * 8, 0:64],
                                pv[:].rearrange("p (h e) -> p h e", e=64))
                        nc.vector.tensor_copy(vr[:, :, 64:65], onesm[:, 0:NH].unsqueeze(2))

            # ------------- Phase B: mean scaling, qT, mean-token k/v -------
            with (
                tc.tile_pool(name="pB", bufs=1) as pB,
                tc.tile_pool(name="pBs", bufs=2) as pBs,
                tc.tile_pool(name="psB", bufs=1, space="PSUM") as psB,
            ):
                meang0 = pB.tile([128, C], F32)
                nc.sync.dma_start(meang0[:], ar1o[0:128, :])
                meang1 = pB.tile([72, C], F32)
                nc.sync.dma_start(meang1[:], ar1o[128:Q, :])
                # w row -> per-partition column. A transposed-view DMA would
                # emit 200 single-element descriptors (~13us on the SP queue,
                # serializing the whole post-AR1 chain); a PE transpose of
                # the contiguous row costs ~1us.
                wrow_g = pB.tile([1, Q], F32)
                nc.sync.dma_start(wrow_g[:], ar1o[Q:Q + 1, 0:Q])
                pt0 = psB.tile([128, 128], F32, tag="tp", bufs=2)
                nc.tensor.transpose(pt0[:, 0:1], wrow_g[0:1, 0:128], ident[0:1, 0:1])
                rw0 = pB.tile([128, 1], F32)
                nc.vector.tensor_scalar_add(rw0[:], pt0[:, 0:1], 0.001)
                nc.vector.reciprocal(rw0[:], rw0[:])
                pt1 = psB.tile([128, 128], F32, tag="tp", bufs=2)
                nc.tensor.transpose(pt1[0:72, 0:1], wrow_g[0:1, 128:Q], ident[0:1, 0:1])
                rw1 = pB.tile([72, 1], F32)
                nc.vector.tensor_scalar_add(rw1[:], pt1[0:72, 0:1], 0.001)
                nc.vector.reciprocal(rw1[:], rw1[:])

                msc0 = pB.tile([128, C], F32)
                nc.vector.tensor_scalar_mul(msc0[:], meang0[:], rw0[:])
                msc1 = pB.tile([72, C], F32)
                nc.vector.tensor_scalar_mul(msc1[:], meang1[:], rw1[:])

                # meanT [c, q]
                meanT = pB.tile([128, 8, Q], MDT)
                for a in range(8):
                    t0 = psB.tile([128, 128], F32, tag="tp", bufs=2)
                    nc.tensor.transpose(t0[:], msc0[:, a * 128:(a + 1) * 128], ident[:])
                    nc.any.tensor_copy(meanT[:, a, 0:128], t0[:])
                    t1 = psB.tile([128, 128], F32, tag="tp", bufs=2)
                    nc.tensor.transpose(t1[:, 0:72], msc1[:, a * 128:(a + 1) * 128], ident[0:72, 0:72])
                    nc.any.tensor_copy(meanT[:, a, 128:Q], t1[:, 0:72])

                # qT and kT over mean tokens
                ktm = pB.tile([128, 8, Q], MDT)
                for a in range(8):
                    pq = psB.tile([128, Q], F32, tag="pq", bufs=2)
                    for kc in range(8):
                        nc.tensor.matmul(pq[:], wq_sb[:, kc, a * 128:(a + 1) * 128],
                                         meanT[:, kc, :],
                                         start=(kc == 0), stop=(kc == 7))
                    nc.any.tensor_scalar_add(qt_sb[:, a, :], pq[:], bq_sb[:, a:a + 1])
                    pk = psB.tile([128, Q], F32, tag="pq", bufs=2)
                    for kc in range(8):
                        nc.tensor.matmul(pk[:], wk_sb[:, kc, a * 128:(a + 1) * 128],
                                         meanT[:, kc, :],
                                         start=(kc == 0), stop=(kc == 7))
                    nc.any.tensor_scalar_add(ktm[:, a, :], pk[:], bk_sb[:, a:a + 1])

                # v over mean tokens, with ones column per head
                vm0 = pB.tile([128, NH * 65], MDT)
                vm1 = pB.tile([72, NH * 65], MDT)
                vm0r = vm0[:].rearrange("p (h e) -> p h e", e=65)
                vm1r = vm1[:].rearrange("p (h e) -> p h e", e=65)
                for (vt, mw, PQC) in ((vm0r, 128, slice(0, 128)), (vm1r, 72, slice(128, Q))):
                    for nn in range(2):
                        pv = psB.tile([128, 512], F32, tag="pq", bufs=2)
                        for kc in range(8):
                            nc.tensor.matmul(pv[0:mw, :], meanT[:, kc, PQC],
                                             wv_sb[:, kc, nn * 512:(nn + 1) * 512],
                                             start=(kc == 0), stop=False)
                        nc.tensor.matmul(pv[0:mw, :], onesm[0:1, 0:mw],
                                         bvr_sb[0:1, nn * 512:(nn + 1) * 512],
                                         start=False, stop=True)
                        nc.any.tensor_copy(
                            vt[:, nn * 8:(nn + 1) * 8, 0:64],
                            pv[0:mw, :].rearrange("p (h e) -> p h e", e=64))
                    nc.vector.tensor_copy(vt[:, :, 64:65], onesm[0:mw, 0:NH].unsqueeze(2))

                # mean-token attention block (all 200 tokens, diag/8 mask).
                # scores go through per-head single-bank PSUM tiles: a shared
                # multi-bank PSUM tile with several matmul groups wedges the
                # device (found empirically).
                for g in range(4):
                    pbm = []
                    for (qc, P, dg) in ((0, 128, diag0), (1, 72, diag1)):
                        pr = pBs.tile([128, 4, Q], MDT, tag="pbm")
                        for hh in range(4):
                            h = 4 * g + hh
                            a, r0 = h // 2, 64 * (h % 2)
                            ps1 = psB.tile([128, Q], F32, tag="ss1", bufs=2)
                            nc.tensor.matmul(
                                ps1[0:P, :],
                                ktm[r0:r0 + 64, a, qc * 128:qc * 128 + P],
                                qt_sb[r0:r0 + 64, a, :],
                                start=True, stop=True)
                            nc.scalar.activation(pr[0:P, hh, :], ps1[0:P, :], AF.Exp, bias=EXP_BIAS)
                        mb = dg[0:P, :].unsqueeze(1).broadcast_to([P, 4, Q])
                        nc.gpsimd.tensor_tensor(pr[0:P, :, :], pr[0:P, :, :], mb, op=ALU.mult)
                        pbm.append(pr)
                    for hh in range(4):
                        h = 4 * g + hh
                        pc2 = psB.tile([65, Q], F32, tag="sctx", bufs=2)
                        nc.tensor.matmul(pc2[:], vm0r[:, h, :], pbm[0][0:128, hh, :], start=True, stop=False)
                        nc.tensor.matmul(pc2[:], vm1r[:, h, :], pbm[1][0:72, hh, :], start=False, stop=True)
                        nc.any.tensor_copy(ctx_sb[:, h, :], pc2[:])

            if phases == 2:
                with tc.tile_pool(name="pX", bufs=1) as pX:
                    ob = pX.tile([128, Q], F32)
                    nc.vector.tensor_copy(ob[:], qt_sb[:, 0, 0:Q])
                    nc.sync.dma_start(outp_d.ap(), ob[:])
                nc.compile()
                return nc

            # ---------------- Phase C: pixel attention --------------------
            with (
                tc.tile_pool(name="pC", bufs=2) as pC,
                tc.tile_pool(name="pCb", bufs=2) as pCb,
                tc.tile_pool(name="psC", bufs=1, space="PSUM") as psC,
            ):
                for dc in range(ndc):
                    if dc < NPRE:
                        kt = kt_pre[dc]
                        vts = [vt_pre[dc][ls][:].rearrange("p (h e) -> p h e", e=65)
                               for ls in range(2)]
                    else:
                        x_dc = pC.tile([128, 8, 256], MDT, tag="xdc")
                        nc.sync.dma_start(x_dc[:], xsr_d.ap()[:, dc, :, :])
                        kt = pC.tile([128, 8, 256], MDT, tag="kt")
                        for a in range(8):
                            pk = psC.tile([128, 256], F32, tag="pkt", bufs=1)
                            for kc in range(8):
                                nc.tensor.matmul(pk[:], wk_sb[:, kc, a * 128:(a + 1) * 128],
                                                 x_dc[:, kc, :],
                                                 start=(kc == 0), stop=(kc == 7))
                            nc.any.tensor_scalar_add(kt[:, a, :], pk[:], bk_sb[:, a:a + 1])

                        vts = []
                        for ls in range(2):
                            vt = pC.tile([128, NH * 65], MDT, tag=f"vt{ls}", bufs=1)
                            vr = vt[:].rearrange("p (h e) -> p h e", e=65)
                            for nn in range(2):
                                pv = psC.tile([128, 512], F32, tag="pv", bufs=2)
                                for kc in range(8):
                                    nc.tensor.matmul(pv[:], x_dc[:, kc, ls * 128:(ls + 1) * 128],
                                                     wv_sb[:, kc, nn * 512:(nn + 1) * 512],
                                                     start=(kc == 0), stop=False)
                                nc.tensor.matmul(pv[:], onesm[0:1, :],
                                                 bvr_sb[0:1, nn * 512:(nn + 1) * 512],
                                                 start=False, stop=True)
                                nc.any.tensor_copy(
                                    vr[:, nn * 8:(nn + 1) * 8, 0:64],
                                    pv[:].rearrange("p (h e) -> p h e", e=64))
                            nc.vector.tensor_copy(vr[:, :, 64:65], onesm[:, 0:NH].unsqueeze(2))
                            vts.append(vr)

                    for g in range(4):
                        pb = []
                        for ls in range(2):
                            pr = pCb.tile([128, 4, Q], MDT, tag="pb")
                            for hh in range(4):
                                h = 4 * g + hh
                                a, r0 = h // 2, 64 * (h % 2)
                                ps1 = psC.tile([128, Q], F32, tag="ss1", bufs=3)
                                nc.tensor.matmul(
                                    ps1[:],
                                    kt[r0:r0 + 64, a, ls * 128:(ls + 1) * 128],
                                    qt_sb[r0:r0 + 64, a, :],
                                    start=True, stop=True)
                                nc.scalar.activation(pr[:, hh, :], ps1[:], AF.Exp, bias=EXP_BIAS)
                            mb = mask01[:, 2 * dc + ls, :].unsqueeze(1).broadcast_to([128, 4, Q])
                            eng = nc.gpsimd if ls == 0 else nc.vector
                            eng.tensor_tensor(pr[:, :, :], pr[:, :, :], mb, op=ALU.mult)
                            pb.append(pr)
                        for hh in range(4):
                            h = 4 * g + hh
                            pc2 = psC.tile([65, Q], F32, tag="sctx", bufs=2)
                            nc.tensor.matmul(pc2[:], vts[0][:, h, :], pb[0][:, hh, :], start=True, stop=False)
                            nc.tensor.matmul(pc2[:], vts[1][:, h, :], pb[1][:, hh, :], start=False, stop=True)
                            nc.vector.tensor_tensor(ctx_sb[:, h, :], ctx_sb[:, h, :], pc2[:], op=ALU.add)

                # ---------------- AR2 + output ----------------------------
                nc.sync.dma_start(
                    ar2i[0:C, :].rearrange("(h p) q -> p h q", p=64),
                    ctx_sb[0:64, :, :])
                # head-sum rows: ctx_sb[64, :, :] is one contiguous [16, 200]
                # block -> single DMA
                nc.sync.dma_start(ar2i[C:C + NH, :], ctx_sb[64:65, :, :])

                nc.gpsimd.collective_compute(
                    "AllReduce", ALU.add, replica_groups=RG,
                    ins=[ar2i.opt()], outs=[ar2o.opt()],
                )

                ctxg = pC.tile([128, 8, Q], F32, bufs=1)
                nc.sync.dma_start(ctxg[:], ar2o[0:C, :].rearrange("(a p) q -> p a q", p=128))
                # softmax denominators in [16, q] partition layout: fast
                # reciprocal, then PE-broadcast each head's row to its 64
                # channel rows via the sel16 indicator matmul
                sums16 = pC.tile([NH, Q], F32, bufs=1)
                nc.sync.dma_start(sums16[:], ar2o[C:C + NH, :])
                rsum16 = pC.tile([NH, Q], MDT, bufs=1)
                with nc.allow_low_precision(reason="softmax denominators; bf16 is the chosen matmul precision"):
                    nc.vector.reciprocal(rsum16[:], sums16[:])
                ctxn = pC.tile([128, 8, Q], MDT, bufs=1)
                for a in range(8):
                    prb = psC.tile([128, 512], F32, tag="pv", bufs=2)
                    nc.tensor.matmul(prb[:, 0:Q], sel16[:, a * 128:(a + 1) * 128],
                                     rsum16[:], start=True, stop=True)
                    nc.vector.tensor_tensor(ctxn[:, a, :], ctxg[:, a, :], prb[:, 0:Q], op=ALU.mult)

                po = psC.tile([128, Q], F32, tag="pkt", bufs=1)
                for kc in range(8):
                    nc.tensor.matmul(po[:], wc_sb[:, kc, :], ctxn[:, kc, :],
                                     start=(kc == 0), stop=(kc == 7))
                outs = pC.tile([128, Q], F32, bufs=1)
                nc.any.tensor_scalar_add(outs[:], po[:], bc_sb[:])
                nc.sync.dma_start(outp_d.ap(), outs[:])

    nc.compile()
    return nc


def make_runner(nc, n_cores=NCORES):
    """Compile nc into a reusable multi-core PJRT callable (compiles once)."""
    import time as _time
    import jax
    from jax.sharding import Mesh, PartitionSpec, NamedSharding
    from jax.experimental.shard_map import shard_map
    from concourse import bass2jax as b2j

    b2j.install_neuronx_cc_hook()

    partition_name = nc.partition_id_tensor.name if nc.partition_id_tensor else None
    in_names, out_names, out_avals, zero_outs = [], [], [], []
    for alloc in nc.m.functions[0].allocations:
        if not isinstance(alloc, mybir.MemoryLocationSet):
            continue
        name = alloc.memorylocations[0].name
        if alloc.kind == "ExternalInput":
            if name != partition_name:
                in_names.append(name)
        elif alloc.kind == "ExternalOutput":
            out_names.append(name)
            shape = tuple(alloc.tensor_shape)
            dtype = mybir.dt.np(alloc.dtype)
            out_avals.append(jax.core.ShapedArray(shape, dtype))
            zero_outs.append(np.zeros(shape, dtype))

    n_params = len(in_names)
    n_outs = len(out_avals)
    all_in_names = in_names + out_names
    if partition_name is not None:
        all_in_names = all_in_names + [partition_name]

    def _body(*args):
        operands = list(args)
        if partition_name is not None:
            operands.append(b2j.partition_id_tensor())
        outs = b2j._bass_exec_p.bind(
            *operands,
            out_avals=tuple(out_avals),
            in_names=tuple(all_in_names),
            out_names=tuple(out_names),
            lowering_input_output_aliases=(),
            sim_require_finite=True,
            sim_require_nnan=True,
            nc=nc,
        )
        return tuple(outs)

    devices = jax.devices()[:n_cores]
    mesh = Mesh(np.asarray(devices), ("core",))
    in_specs = (PartitionSpec("core"),) * (n_params + n_outs)
    out_specs = (PartitionSpec("core"),) * n_outs
    sharded = jax.jit(
        shard_map(_body, mesh=mesh, in_specs=in_specs,
                  out_specs=out_specs, check_rep=False),
        keep_unused=True,
    )
    # Pre-shard args onto the 8 cores. A default device_put would commit
    # everything to one device and force a resharding inside every timed
    # sharded() call.
    arg_sharding = NamedSharding(mesh, PartitionSpec("core"))

    def run(in_maps, iters=0, debug=False):
        concat_in = [
            np.concatenate([np.asarray(in_maps[c][name]) for c in range(n_cores)], axis=0)
            for name in in_names
        ]
        concat_zeros = [np.zeros((n_cores * z.shape[0], *z.shape[1:]), z.dtype)
                        for z in zero_outs]
        t0 = _time.perf_counter()
        args = [jax.device_put(a, arg_sharding) for a in concat_in + concat_zeros]
        jax.block_until_ready(args)
        if debug:
            tot = sum(a.nbytes for a in concat_in)
            print(f"device_put done: {tot/1e6:.0f} MB in {_time.perf_counter()-t0:.1f}s", flush=True)
        out = sharded(*args)
        jax.block_until_ready(out)
        times = []
        for _ in range(iters):
            t0 = _time.perf_counter()
            out2 = sharded(*args)
            jax.block_until_ready(out2)
            times.append(_time.perf_counter() - t0)
        res = [
            {name: np.asarray(out[i]).reshape(n_cores, *out_avals[i].shape)[c]
             for i, name in enumerate(out_names)}
            for c in range(n_cores)
        ]
        return res, times

    return run


_RUNNER = None
_NC = None


def _get_runner():
    global _RUNNER, _NC
    if _RUNNER is None:
        _NC = build()
        _RUNNER = make_runner(_NC)
    return _RUNNER


def make_in_maps(x, masks, Wq, bq, Wk, bk, Wv, bv, Wc, bc):
    f = lambda a: np.ascontiguousarray(np.asarray(a, dtype=np.float32))
    bf = lambda a: np.ascontiguousarray(np.asarray(a).astype(ml_dtypes.bfloat16))
    x, masks = f(x), f(masks)
    Wq, bq, Wk, bk, Wv, bv, Wc, bc = map(f, (Wq, bq, Wk, bk, Wv, bv, Wc, bc))
    X2 = x.reshape(C, HW)
    M2 = masks.reshape(Q, HW)
    s = HD ** -0.5
    WqT = np.ascontiguousarray((Wq * s).T)
    bq_s = f(bq * s)
    WkT = np.ascontiguousarray(Wk.T)
    WvT = np.ascontiguousarray(Wv.T)
    WcT = np.ascontiguousarray(Wc.T)

    def chunked(w):   # [C, N] -> [128, 8, N] with row 128*kc+p -> [p, kc]
        return bf(w.reshape(8, 128, -1).transpose(1, 0, 2))

    wkt_h, wvt_h, wqt_h = chunked(WkT), chunked(WvT), chunked(WqT)
    LOGIT09 = np.float32(np.log(9.0))   # sigmoid(x) > 0.9  <=>  x > ln 9
    diag = np.zeros((Q, Q), np.float32)
    diag[np.arange(Q), np.arange(Q)] = 1.0 / NCORES
    # sel16[h, a*128+p] = 1 iff channel row (a, p) belongs to head h
    sel16 = np.zeros((NH, 8 * 128), np.float32)
    for a in range(8):
        sel16[2 * a, a * 128:a * 128 + 64] = 1.0
        sel16[2 * a + 1, a * 128 + 64:(a + 1) * 128] = 1.0
    onesm = np.ones((128, 128), np.float32)
    in_maps = []
    for c in range(NCORES):
        xc = X2[:, c * LPIX:(c + 1) * LPIX]                    # [C, LPIX]
        # xsr[p, dc, kc, l] = x[128*kc+p, 256*dc+l]
        xsr = bf(xc.reshape(8, 128, NDC, 256).transpose(1, 2, 0, 3))
        # xtr[p, sc, cc] = x[cc, 128*sc+p]
        xtr = xc.reshape(C, NSC, 128).transpose(2, 1, 0)
        mc = M2[:, c * LPIX:(c + 1) * LPIX]                    # [Q, LPIX]
        mskt = np.ascontiguousarray(mc.reshape(Q, NSC, 128).transpose(2, 1, 0))
        m01 = (mskt > LOGIT09).astype(np.float32)
        in_maps.append({
            "xsr": xsr, "xtr": bf(xtr), "mskt": bf(mskt), "m01": bf(m01),
            "wkt": wkt_h, "wvt": wvt_h, "wqt": wqt_h,
            "wct": chunked(np.ascontiguousarray(WcT[:, c * 128:(c + 1) * 128])),
            "bk": np.ascontiguousarray(bk.reshape(8, 128).T),
            "bq": np.ascontiguousarray(bq_s.reshape(8, 128).T),
            "bvr": bf(bv.reshape(1, C)),
            "bc": np.ascontiguousarray(bc[c * 128:(c + 1) * 128].reshape(128, 1)),
            "diag": bf(diag),
            "sel16": bf(sel16), "onesm": bf(onesm),
        })
    return in_maps


def kernel(x, masks, Wq, bq, Wk, bk, Wv, bv, Wc, bc):
    in_maps = make_in_maps(x, masks, Wq, bq, Wk, bk, Wv, bv, Wc, bc)
    run = _get_runner()
    results, _ = run(in_maps)
    outT = np.concatenate([results[c]["outp"] for c in range(NCORES)], axis=0)
    return np.ascontiguousarray(outT.T).reshape(Q, 1, C).astype(np.float32)
